# revision 1
# baseline (speedup 1.0000x reference)
"""Megatron-style MoE layer (precomputed routing) on 8 Trainium2 NeuronCores.

Strategy: expert parallelism. Core e owns expert e's weights (w1[e], w2[e],
resident in SBUF as bf16). The host computes the token->expert routing from
`choosed_experts` (pure index math), gathers each expert's tokens into a
padded, transposed [H, C] activation block, and each core computes

    y_e = coef * (gelu_tanh(x_e @ w1[e]) @ w2[e])

entirely on-device in a features-on-partition layout ([features, tokens]),
so both matmuls use the natural weight layout as lhsT and no on-chip
transposes are needed. The host scatters the per-pair results back and sums
the K=2 contributions per token.

Device layouts (per core), P = 128 partitions:
  xT   [P, 8, C]  bf16   x^T, h = ko*128 + p
  w1   [P, 8, F]  bf16   w1[h, f], h = ko*128 + p  (lhsT for fc1)
  w2   [P, 32, H] bf16   w2[f, hh], f = kf*128 + p (lhsT for fc2)
  coef [P, C]     f32    per-token gate prob, replicated across partitions
  y    [P, 8, C]  f32    y^T, hh = mh*128 + p
"""

import sys
import numpy as np
import ml_dtypes


def _ensure_axon_hooks():
    """bass_utils imports antenv.axon_hooks when BASS_TRACE is set; this
    image ships an antenv stub without it. Provide a working (or None)
    hook so tracing requests degrade gracefully instead of crashing."""
    try:
        import antenv.axon_hooks  # noqa: F401
        return
    except ImportError:
        pass
    import os
    import types

    mod = types.ModuleType("antenv.axon_hooks")
    state = [None]

    def set_axon_ntff_profile_hook(h):
        state[0] = h

    def get_axon_ntff_profile_hook():
        if state[0] is None:
            try:
                from trn_agent_boot.trn_boot import _ntff_profile_via_ctypes
                so = os.environ.get("PJRT_LIBRARY_PATH",
                                    "/opt/axon/libaxon_pjrt.so")
                if os.path.exists(so):
                    state[0] = _ntff_profile_via_ctypes(so)
            except Exception:
                pass
        return state[0]

    mod.set_axon_ntff_profile_hook = set_axon_ntff_profile_hook
    mod.get_axon_ntff_profile_hook = get_axon_ntff_profile_hook
    sys.modules["antenv.axon_hooks"] = mod
    try:
        import antenv
        antenv.axon_hooks = mod
    except ImportError:
        pass
    try:
        from concourse import bass_utils as _bu
        _orig = _bu.upload_artifacts

        def _safe_upload(tmpdir):
            try:
                return _orig(tmpdir)
            except Exception:
                return "local://" + tmpdir

        _bu.upload_artifacts = _safe_upload
    except Exception:
        pass


S, B, H = 1024, 8, 1024
T = S * B
E, K, F = 8, 2, 4096
P = 128
NCORES = 8

_CACHE: dict[int, object] = {}

TRACE = False
LAST_RESULTS = None


def _build(C: int):
    import concourse.bacc as bacc
    import concourse.mybir as mybir
    import concourse.tile as tile

    dt = mybir.dt
    AF = mybir.ActivationFunctionType

    nc = bacc.Bacc("TRN2", target_bir_lowering=False, debug=False,
                   num_devices=NCORES)

    xT_d = nc.dram_tensor("xT", [P, 8, C], dt.bfloat16, kind="ExternalInput").ap()
    w1_d = nc.dram_tensor("w1", [P, 8, F], dt.bfloat16, kind="ExternalInput").ap()
    w2_d = nc.dram_tensor("w2", [P, 32, H], dt.bfloat16, kind="ExternalInput").ap()
    cf_d = nc.dram_tensor("coef", [P, C], dt.float32, kind="ExternalInput").ap()
    y_d = nc.dram_tensor("y", [P, 8, C], dt.float32, kind="ExternalOutput").ap()

    # token tiles of up to 512 (PSUM bank limit for f32 output), sized as
    # evenly as possible (multiples of 32) so every tile stays in the
    # PE streaming regime (N >= 128) instead of one LDWEIGHTS-bound tail
    nt = -(-C // 512)
    sizes = []
    rem = C
    for i in range(nt):
        if i == nt - 1:
            n = rem
        else:
            n = min(512, -(-rem // (nt - i)))
            n = min(512, -(-n // 32) * 32)
            n = min(n, rem)
        sizes.append(n)
        rem -= n
    assert sum(sizes) == C and all(0 < s <= 512 for s in sizes), sizes
    tiles = []
    n0 = 0
    for n in sizes:
        tiles.append((n0, n))
        n0 += n

    with tile.TileContext(nc) as tc:
        with (
            tc.tile_pool(name="wpool", bufs=1) as wpool,
            tc.tile_pool(name="xpool", bufs=2) as xpool,
            tc.tile_pool(name="hpool", bufs=1) as hpool,
            tc.tile_pool(name="opool", bufs=4) as opool,
            tc.tile_pool(name="ps1", bufs=3, space="PSUM") as ps1,
            tc.tile_pool(name="ps2", bufs=3, space="PSUM") as ps2,
        ):
            w1_sb = wpool.tile([P, 8, F], dt.bfloat16, tag="w1")
            w2_sb = wpool.tile([P, 32, H], dt.bfloat16, tag="w2")

            # All sync-engine DMAs share one in-order HWDGE queue, so issue
            # order = completion order. Load the first x tile and w1 first
            # (fc1's critical path), defer w2 until fc1 is underway.
            # The opening cascade is fine-grained and interleaved so the
            # first matmul group (mf=0: w1 f-cols 0:128 + all ko of x)
            # becomes runnable after ~0.6 MB instead of ~1.8 MB.
            N0 = tiles[0][1]
            xt0 = xpool.tile([P, 8, 512], dt.bfloat16, tag="x")
            nc.sync.dma_start(w1_sb[:, :, 0:128], w1_d[:, :, 0:128])
            nc.sync.dma_start(xt0[:, 0:2, :N0], xT_d[:, 0:2, :N0])
            nc.sync.dma_start(xt0[:, 2:4, :N0], xT_d[:, 2:4, :N0])
            nc.sync.dma_start(xt0[:, 4:8, :N0], xT_d[:, 4:8, :N0])
            # rest of w1, coarsening as the PE gets further ahead
            w1_chunks = [(128, 128), (256, 256), (512, 512)] + \
                        [(i * 512, 512) for i in range(2, 8)]
            for (f0, fn) in w1_chunks:
                nc.sync.dma_start(w1_sb[:, :, f0:f0 + fn],
                                  w1_d[:, :, f0:f0 + fn])

            for ti, (t0, N) in enumerate(tiles):
                if ti == 0:
                    xt = xt0
                else:
                    xt = xpool.tile([P, 8, 512], dt.bfloat16, tag="x")
                    nc.sync.dma_start(xt[:, :, :N], xT_d[:, :, t0:t0 + N])
                cf = xpool.tile([P, 512], dt.float32, tag="cf")
                nc.sync.dma_start(cf[:, :N], cf_d[:, t0:t0 + N])

                h = hpool.tile([P, 32, 512], dt.bfloat16, tag="h")
                for mf in range(32):
                    p1 = ps1.tile([P, 512], dt.float32, tag="p1")
                    for ko in range(8):
                        nc.tensor.matmul(
                            p1[:, :N],
                            w1_sb[:, ko, mf * 128:(mf + 1) * 128],
                            xt[:, ko, :N],
                            start=(ko == 0), stop=(ko == 7),
                        )
                    nc.scalar.activation(h[:, mf, :N], p1[:, :N],
                                         AF.Gelu_apprx_tanh)

                if ti == 0:
                    # w2 isn't needed until fc2 of tile 0; issuing it here
                    # keeps it off fc1's DMA critical path
                    for i in range(8):
                        nc.sync.dma_start(w2_sb[:, i * 4:(i + 1) * 4, :],
                                          w2_d[:, i * 4:(i + 1) * 4, :])

                for mh in range(8):
                    p2 = ps2.tile([P, 512], dt.float32, tag="p2")
                    for kf in range(32):
                        nc.tensor.matmul(
                            p2[:, :N],
                            w2_sb[:, kf, mh * 128:(mh + 1) * 128],
                            h[:, kf, :N],
                            start=(kf == 0), stop=(kf == 31),
                        )
                    ot = opool.tile([P, 512], dt.float32, tag="o")
                    nc.vector.tensor_mul(ot[:, :N], p2[:, :N], cf[:, :N])
                    nc.sync.dma_start(y_d[:, mh, t0:t0 + N], ot[:, :N])

    nc.compile()
    return nc


def kernel(hidden_states, gate_weight, choosed_experts, w1, w2):
    global LAST_RESULTS
    _ensure_axon_hooks()
    from concourse import bass_utils

    x = np.asarray(hidden_states, dtype=np.float32).reshape(T, H)
    gw = np.asarray(gate_weight, dtype=np.float32)
    ce = np.asarray(choosed_experts).astype(np.int64)
    w1 = np.asarray(w1, dtype=np.float32)
    w2 = np.asarray(w2, dtype=np.float32)

    # routing: stable sort of (token, k) pairs by expert
    flat = ce.reshape(-1)
    order = np.argsort(flat, kind="stable")
    counts = np.bincount(flat, minlength=E).astype(np.int64)
    starts = np.zeros(E + 1, dtype=np.int64)
    starts[1:] = np.cumsum(counts)

    C = max(512, int(-(-counts.max() // 64)) * 64)

    nc = _CACHE.get(C)
    if nc is None:
        nc = _build(C)
        _CACHE[C] = nc

    bf16 = ml_dtypes.bfloat16
    in_maps = []
    for e in range(E):
        p = order[starts[e]:starts[e + 1]]
        t_idx = p // K
        k_idx = p % K
        n_e = len(p)

        xT = np.zeros((H, C), dtype=bf16)
        xT[:, :n_e] = x[t_idx].T
        xT = np.ascontiguousarray(xT.reshape(8, P, C).transpose(1, 0, 2))

        w1_e = np.ascontiguousarray(
            w1[e].astype(bf16).reshape(8, P, F).transpose(1, 0, 2))
        w2_e = np.ascontiguousarray(
            w2[e].astype(bf16).reshape(32, P, H).transpose(1, 0, 2))

        coef = np.zeros((C,), dtype=np.float32)
        coef[:n_e] = gw[t_idx, k_idx]
        coef = np.ascontiguousarray(np.broadcast_to(coef[None, :], (P, C)))

        in_maps.append({"xT": xT, "w1": w1_e, "w2": w2_e, "coef": coef})

    res = bass_utils.run_bass_kernel_spmd(nc, in_maps, list(range(NCORES)),
                                          trace=TRACE)
    LAST_RESULTS = res

    # combine: place each pair's result back, then sum the K contributions
    ys = []
    for e in range(E):
        y = res.results[e]["y"]  # [P, 8, C] f32
        yT = y.transpose(1, 0, 2).reshape(H, C)
        n_e = int(counts[e])
        ys.append(yT[:, :n_e].T)
    all_pairs = np.concatenate(ys, axis=0)  # [T*K, H] in expert order
    out_pairs = np.empty((T * K, H), dtype=np.float32)
    out_pairs[order] = all_pairs
    return out_pairs.reshape(T, K, H).sum(axis=1)



# revision 3
# speedup vs baseline: 1.0020x; 1.0020x over previous
"""Megatron-style MoE layer (precomputed routing) on 8 Trainium2 NeuronCores.

Strategy: expert parallelism with F-split pairing for load balance. Experts
are paired (largest token count with smallest); the pair's two experts live
on a pair of cores, each core holding HALF of the ffn dimension F of both
experts (w1[:, half], w2[half, :] — 16 MB bf16, SBUF-resident). Both cores
of a pair process the SAME tokens (the union of both experts' dispatched
tokens) through their F-half and emit partial fc2 outputs; the host sums
the two partials. This makes the per-core PE load (count[a]+count[b])/2,
i.e. nearly perfectly balanced, instead of max_e count[e].

Per core, features-on-partition layout ([features, tokens]) so both matmuls
use the natural weight layout as lhsT with no on-chip transposes:

    y_part = coef * (gelu_tanh(x_seg @ w1h[seg]) @ w2h[seg])   seg in {a, b}

Device layouts (per core), P = 128 partitions, F2 = F/2 = 2048:
  xT   [P, 8, CA+CB]    bf16   x^T, h = ko*128 + p (seg a cols [0,CA), b after)
  w1   [P, 2, 8, F2]    bf16   w1[e_seg][h, f_half], lhsT for fc1
  w2   [P, 2, 16, H]    bf16   w2[e_seg][f_half, hh], lhsT for fc2
  coef [P, CA+CB]       bf16   per-token gate prob, replicated across partitions
  y    [P, 8, CA+CB]    bf16   partial y^T, hh = mh*128 + p
"""

import sys
import numpy as np
import ml_dtypes


def _ensure_axon_hooks():
    """bass_utils imports antenv.axon_hooks when BASS_TRACE is set; this
    image ships an antenv stub without it. Provide a working (or None)
    hook so tracing requests degrade gracefully instead of crashing."""
    try:
        import antenv.axon_hooks  # noqa: F401
        return
    except ImportError:
        pass
    import os
    import types

    mod = types.ModuleType("antenv.axon_hooks")
    state = [None]

    def set_axon_ntff_profile_hook(h):
        state[0] = h

    def get_axon_ntff_profile_hook():
        if state[0] is None:
            try:
                from trn_agent_boot.trn_boot import _ntff_profile_via_ctypes
                so = os.environ.get("PJRT_LIBRARY_PATH",
                                    "/opt/axon/libaxon_pjrt.so")
                if os.path.exists(so):
                    state[0] = _ntff_profile_via_ctypes(so)
            except Exception:
                pass
        return state[0]

    mod.set_axon_ntff_profile_hook = set_axon_ntff_profile_hook
    mod.get_axon_ntff_profile_hook = get_axon_ntff_profile_hook
    sys.modules["antenv.axon_hooks"] = mod
    try:
        import antenv
        antenv.axon_hooks = mod
    except ImportError:
        pass
    try:
        from concourse import bass_utils as _bu
        _orig = _bu.upload_artifacts

        def _safe_upload(tmpdir):
            try:
                return _orig(tmpdir)
            except Exception:
                return "local://" + tmpdir

        _bu.upload_artifacts = _safe_upload
    except Exception:
        pass


S, B, H = 1024, 8, 1024
T = S * B
E, K, F = 8, 2, 4096
F2 = F // 2
P = 128
NCORES = 8

_CACHE: dict[tuple, object] = {}

TRACE = False
LAST_RESULTS = None


def _tile_sizes(C: int) -> list[int]:
    """Split C into tiles of at most 512 (PSUM f32 bank limit), all within
    32 of each other so none falls into the LDWEIGHTS-bound regime."""
    assert C % 32 == 0 and C > 0
    nt = -(-C // 512)
    q, r = divmod(C // 32, nt)
    return [(q + 1) * 32] * r + [q * 32] * (nt - r)


def _build(CA: int, CB: int):
    import concourse.bacc as bacc
    import concourse.mybir as mybir
    import concourse.tile as tile

    dt = mybir.dt
    AF = mybir.ActivationFunctionType

    nc = bacc.Bacc("TRN2", target_bir_lowering=False, debug=False,
                   num_devices=NCORES)

    C2 = CA + CB
    xT_d = nc.dram_tensor("xT", [P, 8, C2], dt.bfloat16, kind="ExternalInput").ap()
    w1_d = nc.dram_tensor("w1", [P, 2, 8, F2], dt.bfloat16, kind="ExternalInput").ap()
    w2_d = nc.dram_tensor("w2", [P, 2, 16, H], dt.bfloat16, kind="ExternalInput").ap()
    cf_d = nc.dram_tensor("coef", [P, C2], dt.bfloat16, kind="ExternalInput").ap()
    y_d = nc.dram_tensor("y", [P, 8, C2], dt.bfloat16, kind="ExternalOutput").ap()

    seg_tiles = [_tile_sizes(CA), _tile_sizes(CB)]
    seg_off = [0, CA]

    with tile.TileContext(nc) as tc:
        with (
            tc.tile_pool(name="wpool", bufs=1) as wpool,
            tc.tile_pool(name="xpool", bufs=2) as xpool,
            tc.tile_pool(name="hpool", bufs=1) as hpool,
            tc.tile_pool(name="opool", bufs=4) as opool,
            tc.tile_pool(name="ps1", bufs=3, space="PSUM") as ps1,
            tc.tile_pool(name="ps2", bufs=3, space="PSUM") as ps2,
            tc.tile_pool(name="psw", bufs=1, space="PSUM") as psw,
        ):
            w1_sb = wpool.tile([P, 2, 8, F2], dt.bfloat16, tag="w1")
            w2_sb = wpool.tile([P, 2, 16, H], dt.bfloat16, tag="w2")

            # PE warmup: the HAM clock gate holds the PE at 1.2 GHz until it
            # has been busy ~3.4us. Scratch matmuls (no DMA dependencies) run
            # while the opening DMAs are in flight, so the PE is already at
            # 2.4 GHz when real compute starts.
            warm = wpool.tile([P, 128], dt.bfloat16, tag="warm")
            nc.gpsimd.memset(warm[:, :], 0)
            wp = psw.tile([P, 128], dt.float32, tag="wp")
            NWARM = 26
            for i in range(NWARM):
                nc.tensor.matmul(wp[:, :], warm[:, :], warm[:, :],
                                 start=(i == 0), stop=(i == NWARM - 1))

            # Sync-engine DMAs complete in issue order. Opening cascade:
            # interleave the first w1 column block with the first x tile
            # per-ko so the first matmul group (mf=0 of seg a, tile 0)
            # becomes runnable after ~150 KB instead of ~1.2 MB.
            N0 = seg_tiles[0][0]
            xt0 = xpool.tile([P, 8, 512], dt.bfloat16, tag="x")
            cf0 = xpool.tile([P, 512], dt.bfloat16, tag="cf")
            for ko in range(8):
                nc.sync.dma_start(w1_sb[:, 0, ko, 0:128], w1_d[:, 0, ko, 0:128])
                nc.sync.dma_start(xt0[:, ko, :N0], xT_d[:, ko, 0:N0])
            # rest of w1 seg a, coarsening as the PE gets further ahead
            for (f0, fn) in [(128, 128), (256, 256), (512, 512), (1024, 1024)]:
                nc.sync.dma_start(w1_sb[:, 0, :, f0:f0 + fn],
                                  w1_d[:, 0, :, f0:f0 + fn])
            nc.sync.dma_start(cf0[:, :N0], cf_d[:, 0:N0])
            # w2 seg a: needed when fc2 of tile 0 starts (~24us in)
            for i in range(4):
                nc.sync.dma_start(w2_sb[:, 0, 4 * i:4 * i + 4, :],
                                  w2_d[:, 0, 4 * i:4 * i + 4, :])

            w_late_issued = [False, False]  # w1 seg b, w2 seg b

            for s in range(2):
                for ti, N in enumerate(seg_tiles[s]):
                    t0 = seg_off[s] + sum(seg_tiles[s][:ti])
                    if s == 0 and ti == 0:
                        xt, cf = xt0, cf0
                    else:
                        xt = xpool.tile([P, 8, 512], dt.bfloat16, tag="x")
                        nc.sync.dma_start(xt[:, :, :N], xT_d[:, :, t0:t0 + N])
                        cf = xpool.tile([P, 512], dt.bfloat16, tag="cf")
                        nc.sync.dma_start(cf[:, :N], cf_d[:, t0:t0 + N])

                    h = hpool.tile([P, 16, 512], dt.bfloat16, tag="h")
                    for mf in range(16):
                        p1 = ps1.tile([P, 512], dt.float32, tag="p1")
                        for ko in range(8):
                            nc.tensor.matmul(
                                p1[:, :N],
                                w1_sb[:, s, ko, mf * 128:(mf + 1) * 128],
                                xt[:, ko, :N],
                                start=(ko == 0), stop=(ko == 7),
                            )
                        nc.scalar.activation(h[:, mf, :N], p1[:, :N],
                                             AF.Gelu_apprx_tanh)

                    # stage seg-b weights behind the early seg-a compute,
                    # off tile 0's critical DMA path
                    if s == 0 and not w_late_issued[0] and (
                            ti >= 1 or ti == len(seg_tiles[0]) - 1):
                        for i in range(2):
                            nc.sync.dma_start(
                                w1_sb[:, 1, :, 1024 * i:1024 * (i + 1)],
                                w1_d[:, 1, :, 1024 * i:1024 * (i + 1)])
                        w_late_issued[0] = True
                    elif s == 0 and not w_late_issued[1] and (
                            ti >= 2 or ti == len(seg_tiles[0]) - 1):
                        for i in range(2):
                            nc.sync.dma_start(w2_sb[:, 1, 8 * i:8 * i + 8, :],
                                              w2_d[:, 1, 8 * i:8 * i + 8, :])
                        w_late_issued[1] = True

                    for mh in range(8):
                        p2 = ps2.tile([P, 512], dt.float32, tag="p2")
                        for kf in range(16):
                            nc.tensor.matmul(
                                p2[:, :N],
                                w2_sb[:, s, kf, mh * 128:(mh + 1) * 128],
                                h[:, kf, :N],
                                start=(kf == 0), stop=(kf == 15),
                            )
                        ot = opool.tile([P, 512], dt.bfloat16, tag="o")
                        nc.vector.tensor_mul(ot[:, :N], p2[:, :N], cf[:, :N])
                        nc.sync.dma_start(y_d[:, mh, t0:t0 + N], ot[:, :N])

                # safety for degenerate shapes: make sure seg-b weights are
                # staged before seg b runs
                if s == 0:
                    if not w_late_issued[0]:
                        nc.sync.dma_start(w1_sb[:, 1, :, :], w1_d[:, 1, :, :])
                        w_late_issued[0] = True
                    if not w_late_issued[1]:
                        nc.sync.dma_start(w2_sb[:, 1, :, :], w2_d[:, 1, :, :])
                        w_late_issued[1] = True

    nc.compile()
    return nc


def kernel(hidden_states, gate_weight, choosed_experts, w1, w2):
    global LAST_RESULTS
    _ensure_axon_hooks()
    from concourse import bass_utils

    x = np.asarray(hidden_states, dtype=np.float32).reshape(T, H)
    gw = np.asarray(gate_weight, dtype=np.float32)
    ce = np.asarray(choosed_experts).astype(np.int64)
    w1 = np.asarray(w1, dtype=np.float32)
    w2 = np.asarray(w2, dtype=np.float32)

    # routing: stable sort of (token, k) pairs by expert
    flat = ce.reshape(-1)
    order = np.argsort(flat, kind="stable")
    counts = np.bincount(flat, minlength=E).astype(np.int64)
    starts = np.zeros(E + 1, dtype=np.int64)
    starts[1:] = np.cumsum(counts)

    # pair largest with smallest so each pair's total is near T*K/4
    desc = np.argsort(-counts, kind="stable")
    pairs = [(int(desc[p]), int(desc[7 - p])) for p in range(4)]
    pad32 = lambda n: max(32, -(-int(n) // 32) * 32)
    CA = pad32(counts[desc[0]])
    CB = pad32(counts[desc[4]])

    nc = _CACHE.get((CA, CB))
    if nc is None:
        nc = _build(CA, CB)
        _CACHE[(CA, CB)] = nc

    bf16 = ml_dtypes.bfloat16

    # per-expert gathered activations / coefs (shared by both cores of a pair)
    def gather(e, C):
        p = order[starts[e]:starts[e + 1]]
        t_idx = p // K
        k_idx = p % K
        n_e = len(p)
        xT = np.zeros((H, C), dtype=bf16)
        xT[:, :n_e] = x[t_idx].T
        xT = xT.reshape(8, P, C).transpose(1, 0, 2)
        coef = np.zeros((C,), dtype=np.float32)
        coef[:n_e] = gw[t_idx, k_idx]
        return xT, coef

    in_maps = [None] * NCORES
    for pi, (ea, eb) in enumerate(pairs):
        xa, ca_ = gather(ea, CA)
        xb, cb_ = gather(eb, CB)
        xT = np.ascontiguousarray(np.concatenate([xa, xb], axis=2))
        coef = np.concatenate([ca_, cb_]).astype(bf16)
        coef = np.ascontiguousarray(
            np.broadcast_to(coef[None, :], (P, CA + CB)))
        for half in range(2):
            sl = slice(half * F2, (half + 1) * F2)
            w1h = np.stack([
                w1[e][:, sl].astype(bf16).reshape(8, P, F2).transpose(1, 0, 2)
                for e in (ea, eb)], axis=1)
            w2h = np.stack([
                w2[e][sl, :].astype(bf16).reshape(16, P, H).transpose(1, 0, 2)
                for e in (ea, eb)], axis=1)
            in_maps[2 * pi + half] = {
                "xT": xT, "w1": np.ascontiguousarray(w1h),
                "w2": np.ascontiguousarray(w2h), "coef": coef,
            }

    res = bass_utils.run_bass_kernel_spmd(nc, in_maps, list(range(NCORES)),
                                          trace=TRACE)
    LAST_RESULTS = res

    # combine: sum the two F-half partials per pair, split back per expert,
    # then sum the K contributions per token
    ys = [None] * E
    for pi, (ea, eb) in enumerate(pairs):
        ysum = (res.results[2 * pi]["y"].astype(np.float32)
                + res.results[2 * pi + 1]["y"].astype(np.float32))
        yT = ysum.transpose(1, 0, 2).reshape(H, CA + CB)
        ys[ea] = yT[:, :counts[ea]].T
        ys[eb] = yT[:, CA:CA + counts[eb]].T
    all_pairs = np.concatenate([ys[e] for e in range(E)], axis=0)
    out_pairs = np.empty((T * K, H), dtype=np.float32)
    out_pairs[order] = all_pairs
    return out_pairs.reshape(T, K, H).sum(axis=1)


# revision 4
# speedup vs baseline: 1.0148x; 1.0128x over previous
"""Megatron-style MoE layer (precomputed routing) on 8 Trainium2 NeuronCores.

Strategy: expert parallelism with F-split pairing for load balance. Experts
are paired (largest token count with smallest); the pair's two experts live
on a pair of cores, each core holding HALF of the ffn dimension F of both
experts (w1[:, half], w2[half, :] — 16 MB bf16, SBUF-resident). Both cores
of a pair process the SAME tokens (the union of both experts' dispatched
tokens) through their F-half and emit partial fc2 outputs; the host sums
the two partials. This makes the per-core PE load (count[a]+count[b])/2,
i.e. nearly perfectly balanced, instead of max_e count[e].

Per core, features-on-partition layout ([features, tokens]) so both matmuls
use the natural weight layout as lhsT with no on-chip transposes:

    y_part = coef * (gelu_tanh(x_seg @ w1h[seg]) @ w2h[seg])   seg in {b, a}

Weights are stored pre-blocked into [128, 128] matmul tiles so every weight
DMA moves 2 KB contiguous lines (256 B lines starve the opening cascade).

Device layouts (per core), P = 128 partitions, F2 = F/2 = 2048:
  xT   [P, 8, CB+CA]      bf16  x^T, h = ko*128 + p (seg b cols [0,CB), a after)
  w1   [P, 2, 16, 8, 128] bf16  [p, seg, fb, ko, j] = w1[e_seg][ko*128+p, fb*128+j]
  w2   [P, 2, 8, 16, 128] bf16  [p, seg, mh, kf, j] = w2[e_seg][kf*128+p, mh*128+j]
  coef [P, CB+CA]         bf16  per-token gate prob, replicated across partitions
  y    [P, 8, CB+CA]      bf16  partial y^T, hh = mh*128 + p
"""

import sys
import numpy as np
import ml_dtypes


def _ensure_axon_hooks():
    """bass_utils imports antenv.axon_hooks when BASS_TRACE is set; this
    image ships an antenv stub without it. Provide a working (or None)
    hook so tracing requests degrade gracefully instead of crashing."""
    try:
        import antenv.axon_hooks  # noqa: F401
        return
    except ImportError:
        pass
    import os
    import types

    mod = types.ModuleType("antenv.axon_hooks")
    state = [None]

    def set_axon_ntff_profile_hook(h):
        state[0] = h

    def get_axon_ntff_profile_hook():
        if state[0] is None:
            try:
                from trn_agent_boot.trn_boot import _ntff_profile_via_ctypes
                so = os.environ.get("PJRT_LIBRARY_PATH",
                                    "/opt/axon/libaxon_pjrt.so")
                if os.path.exists(so):
                    state[0] = _ntff_profile_via_ctypes(so)
            except Exception:
                pass
        return state[0]

    mod.set_axon_ntff_profile_hook = set_axon_ntff_profile_hook
    mod.get_axon_ntff_profile_hook = get_axon_ntff_profile_hook
    sys.modules["antenv.axon_hooks"] = mod
    try:
        import antenv
        antenv.axon_hooks = mod
    except ImportError:
        pass
    try:
        from concourse import bass_utils as _bu
        _orig = _bu.upload_artifacts

        def _safe_upload(tmpdir):
            try:
                return _orig(tmpdir)
            except Exception:
                return "local://" + tmpdir

        _bu.upload_artifacts = _safe_upload
    except Exception:
        pass


S, B, H = 1024, 8, 1024
T = S * B
E, K, F = 8, 2, 4096
F2 = F // 2
P = 128
NCORES = 8

_CACHE: dict[tuple, object] = {}

TRACE = False
LAST_RESULTS = None


def _tile_sizes(C: int) -> list[int]:
    """Split C into tiles of at most 512 (PSUM f32 bank limit), all within
    32 of each other (descending) so none is LDWEIGHTS-bound."""
    assert C % 32 == 0 and C > 0
    nt = -(-C // 512)
    q, r = divmod(C // 32, nt)
    return [(q + 1) * 32] * r + [q * 32] * (nt - r)


def _build(CB: int, CA: int):
    import concourse.bacc as bacc
    import concourse.mybir as mybir
    import concourse.tile as tile

    dt = mybir.dt
    AF = mybir.ActivationFunctionType

    nc = bacc.Bacc("TRN2", target_bir_lowering=False, debug=False,
                   num_devices=NCORES)

    C2 = CB + CA
    xT_d = nc.dram_tensor("xT", [P, 8, C2], dt.bfloat16, kind="ExternalInput").ap()
    w1_d = nc.dram_tensor("w1", [P, 2, 16, 8, 128], dt.bfloat16,
                          kind="ExternalInput").ap()
    w2_d = nc.dram_tensor("w2", [P, 2, 8, 16, 128], dt.bfloat16,
                          kind="ExternalInput").ap()
    cf_d = nc.dram_tensor("coef", [P, C2], dt.bfloat16, kind="ExternalInput").ap()
    y_d = nc.dram_tensor("y", [P, 8, C2], dt.bfloat16, kind="ExternalOutput").ap()

    # seg 0 = small expert (starts with the biggest tile: best compute/DMA
    # ratio during the opening ramp), seg 1 = big expert (ends with the
    # smallest tile: shortest drain)
    seg_tiles = [_tile_sizes(CB), _tile_sizes(CA)]
    seg_off = [0, CB]

    with tile.TileContext(nc) as tc:
        with (
            tc.tile_pool(name="wpool", bufs=1) as wpool,
            tc.tile_pool(name="xpool", bufs=2) as xpool,
            tc.tile_pool(name="hpool", bufs=1) as hpool,
            tc.tile_pool(name="opool", bufs=4) as opool,
            tc.tile_pool(name="ps1", bufs=3, space="PSUM") as ps1,
            tc.tile_pool(name="ps2", bufs=3, space="PSUM") as ps2,
            tc.tile_pool(name="psw", bufs=1, space="PSUM") as psw,
        ):
            w1_sb = wpool.tile([P, 2, 16, 8, 128], dt.bfloat16, tag="w1")
            w2_sb = wpool.tile([P, 2, 8, 16, 128], dt.bfloat16, tag="w2")

            # PE warmup: the HAM clock gate holds the PE at 1.2 GHz until it
            # has been busy ~3.4us. Scratch matmuls (no DMA dependencies) run
            # while the opening DMAs are in flight, so the PE is already at
            # 2.4 GHz when real compute starts.
            warm = wpool.tile([P, 128], dt.bfloat16, tag="warm")
            nc.gpsimd.memset(warm[:, :], 0)
            wp = psw.tile([P, 128], dt.float32, tag="wp")
            NWARM = 26
            for i in range(NWARM):
                nc.tensor.matmul(wp[:, :], warm[:, :], warm[:, :],
                                 start=(i == 0), stop=(i == NWARM - 1))

            # Sync-engine DMAs complete in issue order. Opening cascade:
            # first fc1 weight block + the first x tile per-ko, then the
            # remaining seg-0 fc1 weight blocks individually so the DMA
            # stream stays just ahead of the mf-group consumption.
            N0 = seg_tiles[0][0]
            xt0 = xpool.tile([P, 8, 512], dt.bfloat16, tag="x")
            cf0 = xpool.tile([P, 512], dt.bfloat16, tag="cf")
            nc.sync.dma_start(w1_sb[:, 0, 0, :, :], w1_d[:, 0, 0, :, :])
            for ko in range(8):
                nc.sync.dma_start(xt0[:, ko, :N0], xT_d[:, ko, 0:N0])
            for fb in range(1, 16):
                nc.sync.dma_start(w1_sb[:, 0, fb, :, :], w1_d[:, 0, fb, :, :])
            nc.sync.dma_start(cf0[:, :N0], cf_d[:, 0:N0])
            # w2 seg 0 by output block: mh=0 is needed first (fc2 of tile 0)
            for mh in range(8):
                nc.sync.dma_start(w2_sb[:, 0, mh, :, :], w2_d[:, 0, mh, :, :])

            w_late_issued = [False, False]  # w1 seg 1, w2 seg 1

            for s in range(2):
                for ti, N in enumerate(seg_tiles[s]):
                    t0 = seg_off[s] + sum(seg_tiles[s][:ti])
                    if s == 0 and ti == 0:
                        xt, cf = xt0, cf0
                    else:
                        xt = xpool.tile([P, 8, 512], dt.bfloat16, tag="x")
                        nc.sync.dma_start(xt[:, :, :N], xT_d[:, :, t0:t0 + N])
                        cf = xpool.tile([P, 512], dt.bfloat16, tag="cf")
                        nc.sync.dma_start(cf[:, :N], cf_d[:, t0:t0 + N])

                    h = hpool.tile([P, 16, 512], dt.bfloat16, tag="h")
                    for mf in range(16):
                        p1 = ps1.tile([P, 512], dt.float32, tag="p1")
                        for ko in range(8):
                            nc.tensor.matmul(
                                p1[:, :N],
                                w1_sb[:, s, mf, ko, :],
                                xt[:, ko, :N],
                                start=(ko == 0), stop=(ko == 7),
                            )
                        nc.scalar.activation(h[:, mf, :N], p1[:, :N],
                                             AF.Gelu_apprx_tanh)

                    # stage seg-1 weights behind the early seg-0 compute,
                    # off tile 0's critical DMA path
                    if s == 0 and not w_late_issued[0] and (
                            ti >= 1 or ti == len(seg_tiles[0]) - 1):
                        for i in range(4):
                            nc.sync.dma_start(
                                w1_sb[:, 1, 4 * i:4 * i + 4, :, :],
                                w1_d[:, 1, 4 * i:4 * i + 4, :, :])
                        w_late_issued[0] = True
                    elif s == 0 and not w_late_issued[1] and (
                            ti >= 2 or ti == len(seg_tiles[0]) - 1):
                        for i in range(4):
                            nc.sync.dma_start(
                                w2_sb[:, 1, 2 * i:2 * i + 2, :, :],
                                w2_d[:, 1, 2 * i:2 * i + 2, :, :])
                        w_late_issued[1] = True

                    for mh in range(8):
                        p2 = ps2.tile([P, 512], dt.float32, tag="p2")
                        for kf in range(16):
                            nc.tensor.matmul(
                                p2[:, :N],
                                w2_sb[:, s, mh, kf, :],
                                h[:, kf, :N],
                                start=(kf == 0), stop=(kf == 15),
                            )
                        ot = opool.tile([P, 512], dt.bfloat16, tag="o")
                        nc.vector.tensor_mul(ot[:, :N], p2[:, :N], cf[:, :N])
                        nc.sync.dma_start(y_d[:, mh, t0:t0 + N], ot[:, :N])

                # safety for degenerate shapes: make sure seg-1 weights are
                # staged before seg 1 runs
                if s == 0:
                    if not w_late_issued[0]:
                        nc.sync.dma_start(w1_sb[:, 1], w1_d[:, 1])
                        w_late_issued[0] = True
                    if not w_late_issued[1]:
                        nc.sync.dma_start(w2_sb[:, 1], w2_d[:, 1])
                        w_late_issued[1] = True

    nc.compile()
    return nc


def kernel(hidden_states, gate_weight, choosed_experts, w1, w2):
    global LAST_RESULTS
    _ensure_axon_hooks()
    from concourse import bass_utils

    x = np.asarray(hidden_states, dtype=np.float32).reshape(T, H)
    gw = np.asarray(gate_weight, dtype=np.float32)
    ce = np.asarray(choosed_experts).astype(np.int64)
    w1 = np.asarray(w1, dtype=np.float32)
    w2 = np.asarray(w2, dtype=np.float32)

    # routing: stable sort of (token, k) pairs by expert
    flat = ce.reshape(-1)
    order = np.argsort(flat, kind="stable")
    counts = np.bincount(flat, minlength=E).astype(np.int64)
    starts = np.zeros(E + 1, dtype=np.int64)
    starts[1:] = np.cumsum(counts)

    # pair largest with smallest so each pair's total is near T*K/4
    desc = np.argsort(-counts, kind="stable")
    pairs = [(int(desc[p]), int(desc[7 - p])) for p in range(4)]
    pad32 = lambda n: max(32, -(-int(n) // 32) * 32)
    CA = pad32(counts[desc[0]])
    CB = pad32(counts[desc[4]])

    nc = _CACHE.get((CB, CA))
    if nc is None:
        nc = _build(CB, CA)
        _CACHE[(CB, CA)] = nc

    bf16 = ml_dtypes.bfloat16

    # per-expert gathered activations / coefs (shared by both cores of a pair)
    def gather(e, C):
        p = order[starts[e]:starts[e + 1]]
        t_idx = p // K
        k_idx = p % K
        n_e = len(p)
        xT = np.zeros((H, C), dtype=bf16)
        xT[:, :n_e] = x[t_idx].T
        xT = xT.reshape(8, P, C).transpose(1, 0, 2)
        coef = np.zeros((C,), dtype=np.float32)
        coef[:n_e] = gw[t_idx, k_idx]
        return xT, coef

    in_maps = [None] * NCORES
    for pi, (ea, eb) in enumerate(pairs):
        xa, ca_ = gather(ea, CA)
        xb, cb_ = gather(eb, CB)
        # seg 0 = small expert (b), seg 1 = big expert (a)
        xT = np.ascontiguousarray(np.concatenate([xb, xa], axis=2))
        coef = np.concatenate([cb_, ca_]).astype(bf16)
        coef = np.ascontiguousarray(
            np.broadcast_to(coef[None, :], (P, CB + CA)))
        for half in range(2):
            sl = slice(half * F2, (half + 1) * F2)
            # w1 blocked [p, seg, fb, ko, j]; w2 blocked [p, seg, mh, kf, j]
            w1h = np.stack([
                w1[e][:, sl].astype(bf16).reshape(8, P, 16, 128)
                .transpose(1, 2, 0, 3)
                for e in (eb, ea)], axis=1)
            w2h = np.stack([
                w2[e][sl, :].astype(bf16).reshape(16, P, 8, 128)
                .transpose(1, 2, 0, 3)
                for e in (eb, ea)], axis=1)
            in_maps[2 * pi + half] = {
                "xT": xT, "w1": np.ascontiguousarray(w1h),
                "w2": np.ascontiguousarray(w2h), "coef": coef,
            }

    res = bass_utils.run_bass_kernel_spmd(nc, in_maps, list(range(NCORES)),
                                          trace=TRACE)
    LAST_RESULTS = res

    # combine: sum the two F-half partials per pair, split back per expert,
    # then sum the K contributions per token
    ys = [None] * E
    for pi, (ea, eb) in enumerate(pairs):
        ysum = (res.results[2 * pi]["y"].astype(np.float32)
                + res.results[2 * pi + 1]["y"].astype(np.float32))
        yT = ysum.transpose(1, 0, 2).reshape(H, CB + CA)
        ys[eb] = yT[:, :counts[eb]].T
        ys[ea] = yT[:, CB:CB + counts[ea]].T
    all_pairs = np.concatenate([ys[e] for e in range(E)], axis=0)
    out_pairs = np.empty((T * K, H), dtype=np.float32)
    out_pairs[order] = all_pairs
    return out_pairs.reshape(T, K, H).sum(axis=1)


# revision 7
# speedup vs baseline: 1.0212x; 1.0063x over previous
"""Megatron-style MoE layer (precomputed routing) on 8 Trainium2 NeuronCores.

Strategy: expert parallelism with F-split pairing for load balance. Experts
are paired (largest token count with smallest); the pair's two experts live
on a pair of cores, each core holding HALF of the ffn dimension F of both
experts (w1[:, half], w2[half, :] — 16 MB bf16, SBUF-resident). Both cores
of a pair process the SAME tokens (the union of both experts' dispatched
tokens) through their F-half and emit partial fc2 outputs; the host sums
the two partials. This makes the per-core PE load (count[a]+count[b])/2,
i.e. nearly perfectly balanced, instead of max_e count[e].

Per core, features-on-partition layout ([features, tokens]) so both matmuls
use the natural weight layout as lhsT with no on-chip transposes:

    y_part = coef * (gelu_tanh(x_seg @ w1h[seg]) @ w2h[seg])   seg in {b, a}

Weights are stored pre-blocked into [128, 128] matmul tiles so every weight
DMA moves 2 KB contiguous lines (256 B lines starve the opening cascade).

Device layouts (per core), P = 128 partitions, F2 = F/2 = 2048:
  xT   [P, 8, CB+CA]      bf16  x^T, h = ko*128 + p (seg b cols [0,CB), a after)
  w1   [P, 2, 16, 8, 128] bf16  [p, seg, fb, ko, j] = w1[e_seg][ko*128+p, fb*128+j]
  w2   [P, 2, 8, 16, 128] bf16  [p, seg, mh, kf, j] = w2[e_seg][kf*128+p, mh*128+j]
  coef [P, CB+CA]         bf16  per-token gate prob, replicated across partitions
  y    [P, 8, CB+CA]      bf16  partial y^T, hh = mh*128 + p
"""

import sys
import numpy as np
import ml_dtypes


def _ensure_axon_hooks():
    """bass_utils imports antenv.axon_hooks when BASS_TRACE is set; this
    image ships an antenv stub without it. Provide a working (or None)
    hook so tracing requests degrade gracefully instead of crashing."""
    try:
        import antenv.axon_hooks  # noqa: F401
        return
    except ImportError:
        pass
    import os
    import types

    mod = types.ModuleType("antenv.axon_hooks")
    state = [None]

    def set_axon_ntff_profile_hook(h):
        state[0] = h

    def get_axon_ntff_profile_hook():
        if state[0] is None:
            try:
                from trn_agent_boot.trn_boot import _ntff_profile_via_ctypes
                so = os.environ.get("PJRT_LIBRARY_PATH",
                                    "/opt/axon/libaxon_pjrt.so")
                if os.path.exists(so):
                    state[0] = _ntff_profile_via_ctypes(so)
            except Exception:
                pass
        return state[0]

    mod.set_axon_ntff_profile_hook = set_axon_ntff_profile_hook
    mod.get_axon_ntff_profile_hook = get_axon_ntff_profile_hook
    sys.modules["antenv.axon_hooks"] = mod
    try:
        import antenv
        antenv.axon_hooks = mod
    except ImportError:
        pass
    try:
        from concourse import bass_utils as _bu
        _orig = _bu.upload_artifacts

        def _safe_upload(tmpdir):
            try:
                return _orig(tmpdir)
            except Exception:
                return "local://" + tmpdir

        _bu.upload_artifacts = _safe_upload
    except Exception:
        pass


S, B, H = 1024, 8, 1024
T = S * B
E, K, F = 8, 2, 4096
F2 = F // 2
P = 128
NCORES = 8

_CACHE: dict[tuple, object] = {}

TRACE = False
LAST_RESULTS = None


def _tile_sizes(C: int) -> list[int]:
    """Split C into tiles of at most 512 (PSUM f32 bank limit), all within
    32 of each other (descending) so none is LDWEIGHTS-bound."""
    assert C % 32 == 0 and C > 0
    nt = -(-C // 512)
    q, r = divmod(C // 32, nt)
    return [(q + 1) * 32] * r + [q * 32] * (nt - r)


def _build(CB: int, CA: int):
    import concourse.bacc as bacc
    import concourse.mybir as mybir
    import concourse.tile as tile

    dt = mybir.dt
    AF = mybir.ActivationFunctionType

    nc = bacc.Bacc("TRN2", target_bir_lowering=False, debug=False,
                   num_devices=NCORES)

    C2 = CB + CA
    xT_d = nc.dram_tensor("xT", [P, 8, C2], dt.bfloat16, kind="ExternalInput").ap()
    w1_d = nc.dram_tensor("w1", [P, 2, 16, 8, 128], dt.bfloat16,
                          kind="ExternalInput").ap()
    w2_d = nc.dram_tensor("w2", [P, 2, 8, 16, 128], dt.bfloat16,
                          kind="ExternalInput").ap()
    cf_d = nc.dram_tensor("coef", [P, C2], dt.bfloat16, kind="ExternalInput").ap()
    y_d = nc.dram_tensor("y", [P, 8, C2], dt.bfloat16, kind="ExternalOutput").ap()

    # seg 0 = small expert (starts with the biggest tile: best compute/DMA
    # ratio during the opening ramp), seg 1 = big expert (ends with the
    # smallest tile: shortest drain)
    seg_tiles = [_tile_sizes(CB), _tile_sizes(CA)]
    seg_off = [0, CB]

    with tile.TileContext(nc) as tc:
        with (
            tc.tile_pool(name="wpool", bufs=1) as wpool,
            tc.tile_pool(name="xpool", bufs=2) as xpool,
            tc.tile_pool(name="hpool", bufs=1) as hpool,
            tc.tile_pool(name="opool", bufs=4) as opool,
            tc.tile_pool(name="ps1", bufs=3, space="PSUM") as ps1,
            tc.tile_pool(name="ps2", bufs=3, space="PSUM") as ps2,
            tc.tile_pool(name="psw", bufs=1, space="PSUM") as psw,
        ):
            w1_sb = wpool.tile([P, 2, 16, 8, 128], dt.bfloat16, tag="w1")
            w2_sb = wpool.tile([P, 2, 8, 16, 128], dt.bfloat16, tag="w2")

            # PE warmup: the HAM clock gate holds the PE at 1.2 GHz until it
            # has been busy ~3.4us. Scratch matmuls (no DMA dependencies) run
            # while the opening DMAs are in flight, so the PE is already at
            # 2.4 GHz when real compute starts.
            # Init the scratch on the vector engine (earliest BB entry of the
            # idle engines — gpsimd took ~1.5us longer to come up).
            warm = wpool.tile([P, 128], dt.bfloat16, tag="warm")
            nc.vector.memset(warm[:, :], 0)
            wp = psw.tile([P, 128], dt.float32, tag="wp")
            NWARM = 34
            for i in range(NWARM):
                nc.tensor.matmul(wp[:, :], warm[:, :], warm[:, :],
                                 start=(i == 0), stop=(i == NWARM - 1))

            # Sync-engine DMAs complete in issue order, and each dma_start
            # costs ~0.5us of sync-engine issue time — keep the upfront
            # issue count minimal, then stream the remaining seg-0 fc1
            # weight blocks individually just ahead of mf-group consumption.
            N0 = seg_tiles[0][0]
            xt0 = xpool.tile([P, 8, 512], dt.bfloat16, tag="x")
            cf0 = xpool.tile([P, 512], dt.bfloat16, tag="cf")
            nc.sync.dma_start(w1_sb[:, 0, 0, :, :], w1_d[:, 0, 0, :, :])
            nc.sync.dma_start(xt0[:, 0:4, :N0], xT_d[:, 0:4, 0:N0])
            nc.sync.dma_start(xt0[:, 4:8, :N0], xT_d[:, 4:8, 0:N0])
            for fb in range(1, 16):
                nc.sync.dma_start(w1_sb[:, 0, fb, :, :], w1_d[:, 0, fb, :, :])
            nc.sync.dma_start(cf0[:, :N0], cf_d[:, 0:N0])
            # w2 seg 0 by output block pairs: mh=0 is needed first (fc2 of
            # tile 0)
            for i in range(4):
                nc.sync.dma_start(w2_sb[:, 0, 2 * i:2 * i + 2, :, :],
                                  w2_d[:, 0, 2 * i:2 * i + 2, :, :])

            w_late_issued = [False, False]  # w1 seg 1, w2 seg 1

            for s in range(2):
                for ti, N in enumerate(seg_tiles[s]):
                    t0 = seg_off[s] + sum(seg_tiles[s][:ti])
                    if s == 0 and ti == 0:
                        xt, cf = xt0, cf0
                    else:
                        xt = xpool.tile([P, 8, 512], dt.bfloat16, tag="x")
                        nc.sync.dma_start(xt[:, :, :N], xT_d[:, :, t0:t0 + N])
                        cf = xpool.tile([P, 512], dt.bfloat16, tag="cf")
                        nc.sync.dma_start(cf[:, :N], cf_d[:, t0:t0 + N])

                    h = hpool.tile([P, 16, 512], dt.bfloat16, tag="h")
                    for mf in range(16):
                        p1 = ps1.tile([P, 512], dt.float32, tag="p1")
                        for ko in range(8):
                            nc.tensor.matmul(
                                p1[:, :N],
                                w1_sb[:, s, mf, ko, :],
                                xt[:, ko, :N],
                                start=(ko == 0), stop=(ko == 7),
                            )
                        nc.scalar.activation(h[:, mf, :N], p1[:, :N],
                                             AF.Gelu_apprx_tanh)

                    # stage seg-1 weights behind the early seg-0 compute,
                    # off tile 0's critical DMA path
                    if s == 0 and not w_late_issued[0] and (
                            ti >= 1 or ti == len(seg_tiles[0]) - 1):
                        for i in range(2):
                            nc.sync.dma_start(
                                w1_sb[:, 1, 8 * i:8 * i + 8, :, :],
                                w1_d[:, 1, 8 * i:8 * i + 8, :, :])
                        w_late_issued[0] = True
                    elif s == 0 and not w_late_issued[1] and (
                            ti >= 2 or ti == len(seg_tiles[0]) - 1):
                        for i in range(2):
                            nc.sync.dma_start(
                                w2_sb[:, 1, 4 * i:4 * i + 4, :, :],
                                w2_d[:, 1, 4 * i:4 * i + 4, :, :])
                        w_late_issued[1] = True

                    last_tile = (s == 1 and ti == len(seg_tiles[1]) - 1)
                    for mh in range(8):
                        if last_tile and mh == 7:
                            # split the final group in two so the last
                            # multiply+DMA overlaps the second half's matmuls
                            nh = ((N // 2) + 31) // 32 * 32
                            for (c0, cn) in ((0, nh), (nh, N - nh)):
                                p2 = ps2.tile([P, 512], dt.float32, tag="p2")
                                for kf in range(16):
                                    nc.tensor.matmul(
                                        p2[:, :cn],
                                        w2_sb[:, s, mh, kf, :],
                                        h[:, kf, c0:c0 + cn],
                                        start=(kf == 0), stop=(kf == 15),
                                    )
                                ot = opool.tile([P, 512], dt.bfloat16, tag="o")
                                nc.vector.tensor_mul(ot[:, :cn], p2[:, :cn],
                                                     cf[:, c0:c0 + cn])
                                nc.sync.dma_start(
                                    y_d[:, mh, t0 + c0:t0 + c0 + cn],
                                    ot[:, :cn])
                            continue
                        p2 = ps2.tile([P, 512], dt.float32, tag="p2")
                        for kf in range(16):
                            nc.tensor.matmul(
                                p2[:, :N],
                                w2_sb[:, s, mh, kf, :],
                                h[:, kf, :N],
                                start=(kf == 0), stop=(kf == 15),
                            )
                        ot = opool.tile([P, 512], dt.bfloat16, tag="o")
                        nc.vector.tensor_mul(ot[:, :N], p2[:, :N], cf[:, :N])
                        nc.sync.dma_start(y_d[:, mh, t0:t0 + N], ot[:, :N])

                # safety for degenerate shapes: make sure seg-1 weights are
                # staged before seg 1 runs
                if s == 0:
                    if not w_late_issued[0]:
                        nc.sync.dma_start(w1_sb[:, 1], w1_d[:, 1])
                        w_late_issued[0] = True
                    if not w_late_issued[1]:
                        nc.sync.dma_start(w2_sb[:, 1], w2_d[:, 1])
                        w_late_issued[1] = True

    nc.compile()
    return nc


def kernel(hidden_states, gate_weight, choosed_experts, w1, w2):
    global LAST_RESULTS
    _ensure_axon_hooks()
    from concourse import bass_utils

    x = np.asarray(hidden_states, dtype=np.float32).reshape(T, H)
    gw = np.asarray(gate_weight, dtype=np.float32)
    ce = np.asarray(choosed_experts).astype(np.int64)
    w1 = np.asarray(w1, dtype=np.float32)
    w2 = np.asarray(w2, dtype=np.float32)

    # routing: stable sort of (token, k) pairs by expert
    flat = ce.reshape(-1)
    order = np.argsort(flat, kind="stable")
    counts = np.bincount(flat, minlength=E).astype(np.int64)
    starts = np.zeros(E + 1, dtype=np.int64)
    starts[1:] = np.cumsum(counts)

    # pair largest with smallest so each pair's total is near T*K/4
    desc = np.argsort(-counts, kind="stable")
    pairs = [(int(desc[p]), int(desc[7 - p])) for p in range(4)]
    pad32 = lambda n: max(32, -(-int(n) // 32) * 32)
    CA = pad32(counts[desc[0]])
    CB = pad32(counts[desc[4]])

    nc = _CACHE.get((CB, CA))
    if nc is None:
        nc = _build(CB, CA)
        _CACHE[(CB, CA)] = nc

    bf16 = ml_dtypes.bfloat16

    # per-expert gathered activations / coefs (shared by both cores of a pair)
    def gather(e, C):
        p = order[starts[e]:starts[e + 1]]
        t_idx = p // K
        k_idx = p % K
        n_e = len(p)
        xT = np.zeros((H, C), dtype=bf16)
        xT[:, :n_e] = x[t_idx].T
        xT = xT.reshape(8, P, C).transpose(1, 0, 2)
        coef = np.zeros((C,), dtype=np.float32)
        coef[:n_e] = gw[t_idx, k_idx]
        return xT, coef

    in_maps = [None] * NCORES
    for pi, (ea, eb) in enumerate(pairs):
        xa, ca_ = gather(ea, CA)
        xb, cb_ = gather(eb, CB)
        # seg 0 = small expert (b), seg 1 = big expert (a)
        xT = np.ascontiguousarray(np.concatenate([xb, xa], axis=2))
        coef = np.concatenate([cb_, ca_]).astype(bf16)
        coef = np.ascontiguousarray(
            np.broadcast_to(coef[None, :], (P, CB + CA)))
        for half in range(2):
            sl = slice(half * F2, (half + 1) * F2)
            # w1 blocked [p, seg, fb, ko, j]; w2 blocked [p, seg, mh, kf, j]
            w1h = np.stack([
                w1[e][:, sl].astype(bf16).reshape(8, P, 16, 128)
                .transpose(1, 2, 0, 3)
                for e in (eb, ea)], axis=1)
            w2h = np.stack([
                w2[e][sl, :].astype(bf16).reshape(16, P, 8, 128)
                .transpose(1, 2, 0, 3)
                for e in (eb, ea)], axis=1)
            in_maps[2 * pi + half] = {
                "xT": xT, "w1": np.ascontiguousarray(w1h),
                "w2": np.ascontiguousarray(w2h), "coef": coef,
            }

    res = bass_utils.run_bass_kernel_spmd(nc, in_maps, list(range(NCORES)),
                                          trace=TRACE)
    LAST_RESULTS = res

    # combine: sum the two F-half partials per pair, split back per expert,
    # then sum the K contributions per token
    ys = [None] * E
    for pi, (ea, eb) in enumerate(pairs):
        ysum = (res.results[2 * pi]["y"].astype(np.float32)
                + res.results[2 * pi + 1]["y"].astype(np.float32))
        yT = ysum.transpose(1, 0, 2).reshape(H, CB + CA)
        ys[eb] = yT[:, :counts[eb]].T
        ys[ea] = yT[:, CB:CB + counts[ea]].T
    all_pairs = np.concatenate([ys[e] for e in range(E)], axis=0)
    out_pairs = np.empty((T * K, H), dtype=np.float32)
    out_pairs[order] = all_pairs
    return out_pairs.reshape(T, K, H).sum(axis=1)


# revision 9
# speedup vs baseline: 1.0230x; 1.0018x over previous
"""Megatron-style MoE layer (precomputed routing) on 8 Trainium2 NeuronCores.

Strategy: expert parallelism with F-split pairing for load balance. Experts
are paired (largest token count with smallest); the pair's two experts live
on a pair of cores, each core holding HALF of the ffn dimension F of both
experts (w1[:, half], w2[half, :] — 16 MB bf16, SBUF-resident). Both cores
of a pair process the SAME tokens (the union of both experts' dispatched
tokens) through their F-half and emit partial fc2 outputs; the host sums
the two partials. This makes the per-core PE load (count[a]+count[b])/2,
i.e. nearly perfectly balanced, instead of max_e count[e].

Per core, features-on-partition layout ([features, tokens]) so both matmuls
use the natural weight layout as lhsT with no on-chip transposes:

    y_part = coef * (gelu_tanh(x_seg @ w1h[seg]) @ w2h[seg])   seg in {b, a}

Weights are stored pre-blocked into [128, 128] matmul tiles so every weight
DMA moves 2 KB contiguous lines (256 B lines starve the opening cascade).

Device layouts (per core), P = 128 partitions, F2 = F/2 = 2048:
  xT   [P, 8, CB+CA]      bf16  x^T, h = ko*128 + p (seg b cols [0,CB), a after)
  w1   [P, 2, 16, 8, 128] bf16  [p, seg, fb, ko, j] = w1[e_seg][ko*128+p, fb*128+j]
  w2   [P, 2, 8, 16, 128] bf16  [p, seg, mh, kf, j] = w2[e_seg][kf*128+p, mh*128+j]
  coef [P, CB+CA]         bf16  per-token gate prob, replicated across partitions
  y    [P, 8, CB+CA]      bf16  partial y^T, hh = mh*128 + p
"""

import sys
import numpy as np
import ml_dtypes


def _ensure_axon_hooks():
    """bass_utils imports antenv.axon_hooks when BASS_TRACE is set; this
    image ships an antenv stub without it. Provide a working (or None)
    hook so tracing requests degrade gracefully instead of crashing."""
    try:
        import antenv.axon_hooks  # noqa: F401
        return
    except ImportError:
        pass
    import os
    import types

    mod = types.ModuleType("antenv.axon_hooks")
    state = [None]

    def set_axon_ntff_profile_hook(h):
        state[0] = h

    def get_axon_ntff_profile_hook():
        if state[0] is None:
            try:
                from trn_agent_boot.trn_boot import _ntff_profile_via_ctypes
                so = os.environ.get("PJRT_LIBRARY_PATH",
                                    "/opt/axon/libaxon_pjrt.so")
                if os.path.exists(so):
                    state[0] = _ntff_profile_via_ctypes(so)
            except Exception:
                pass
        return state[0]

    mod.set_axon_ntff_profile_hook = set_axon_ntff_profile_hook
    mod.get_axon_ntff_profile_hook = get_axon_ntff_profile_hook
    sys.modules["antenv.axon_hooks"] = mod
    try:
        import antenv
        antenv.axon_hooks = mod
    except ImportError:
        pass
    try:
        from concourse import bass_utils as _bu
        _orig = _bu.upload_artifacts

        def _safe_upload(tmpdir):
            try:
                return _orig(tmpdir)
            except Exception:
                return "local://" + tmpdir

        _bu.upload_artifacts = _safe_upload
    except Exception:
        pass


S, B, H = 1024, 8, 1024
T = S * B
E, K, F = 8, 2, 4096
F2 = F // 2
P = 128
NCORES = 8

_CACHE: dict[tuple, object] = {}

TRACE = False
LAST_RESULTS = None


def _tile_sizes(C: int) -> list[int]:
    """Split C into tiles of at most 512 (PSUM f32 bank limit), all within
    32 of each other (descending) so none is LDWEIGHTS-bound."""
    assert C % 32 == 0 and C > 0
    nt = -(-C // 512)
    q, r = divmod(C // 32, nt)
    return [(q + 1) * 32] * r + [q * 32] * (nt - r)


def _build(CB: int, CA: int):
    import concourse.bacc as bacc
    import concourse.mybir as mybir
    import concourse.tile as tile

    dt = mybir.dt
    AF = mybir.ActivationFunctionType

    nc = bacc.Bacc("TRN2", target_bir_lowering=False, debug=False,
                   num_devices=NCORES)

    C2 = CB + CA
    xT_d = nc.dram_tensor("xT", [P, 8, C2], dt.bfloat16, kind="ExternalInput").ap()
    w1_d = nc.dram_tensor("w1", [P, 2, 16, 8, 128], dt.bfloat16,
                          kind="ExternalInput").ap()
    w2_d = nc.dram_tensor("w2", [P, 2, 8, 16, 128], dt.bfloat16,
                          kind="ExternalInput").ap()
    cf_d = nc.dram_tensor("coef", [P, C2], dt.bfloat16, kind="ExternalInput").ap()
    y_d = nc.dram_tensor("y", [P, 8, C2], dt.bfloat16, kind="ExternalOutput").ap()

    # seg 0 = small expert (starts with the biggest tile: best compute/DMA
    # ratio during the opening ramp), seg 1 = big expert (ends with the
    # smallest tile: shortest drain)
    seg_tiles = [_tile_sizes(CB), _tile_sizes(CA)]
    seg_off = [0, CB]

    with tile.TileContext(nc) as tc:
        with (
            tc.tile_pool(name="wpool", bufs=1) as wpool,
            tc.tile_pool(name="xpool", bufs=2) as xpool,
            tc.tile_pool(name="hpool", bufs=1) as hpool,
            tc.tile_pool(name="opool", bufs=4) as opool,
            tc.tile_pool(name="ps1", bufs=3, space="PSUM") as ps1,
            tc.tile_pool(name="ps2", bufs=3, space="PSUM") as ps2,
            tc.tile_pool(name="psw", bufs=1, space="PSUM") as psw,
        ):
            w1_sb = wpool.tile([P, 2, 16, 8, 128], dt.bfloat16, tag="w1")
            w2_sb = wpool.tile([P, 2, 8, 16, 128], dt.bfloat16, tag="w2")

            # PE warmup: the HAM clock gate holds the PE at 1.2 GHz until it
            # has been busy ~3.4us. Scratch matmuls (no DMA dependencies) run
            # while the opening DMAs are in flight, so the PE is already at
            # 2.4 GHz when real compute starts.
            # Init the scratch on the vector engine (earliest BB entry of the
            # idle engines — gpsimd took ~1.5us longer to come up).
            warm = wpool.tile([P, 128], dt.bfloat16, tag="warm")
            nc.vector.memset(warm[:, :], 0)
            wp = psw.tile([P, 128], dt.float32, tag="wp")
            NWARM = 38
            for i in range(NWARM):
                nc.tensor.matmul(wp[:, :], warm[:, :], warm[:, :],
                                 start=(i == 0), stop=(i == NWARM - 1))

            # Sync-engine DMAs complete in issue order, and each dma_start
            # costs ~0.5us of sync-engine issue time — keep the upfront
            # issue count minimal, then stream the remaining seg-0 fc1
            # weight blocks individually just ahead of mf-group consumption.
            N0 = seg_tiles[0][0]
            xt0 = xpool.tile([P, 8, 512], dt.bfloat16, tag="x")
            cf0 = xpool.tile([P, 512], dt.bfloat16, tag="cf")
            nc.sync.dma_start(w1_sb[:, 0, 0, :, :], w1_d[:, 0, 0, :, :])
            nc.sync.dma_start(xt0[:, 0:4, :N0], xT_d[:, 0:4, 0:N0])
            nc.sync.dma_start(w1_sb[:, 0, 1, :, :], w1_d[:, 0, 1, :, :])
            nc.sync.dma_start(xt0[:, 4:8, :N0], xT_d[:, 4:8, 0:N0])
            for fb in range(2, 16):
                nc.sync.dma_start(w1_sb[:, 0, fb, :, :], w1_d[:, 0, fb, :, :])
            nc.sync.dma_start(cf0[:, :N0], cf_d[:, 0:N0])
            # w2 seg 0 by output block pairs: mh=0 is needed first (fc2 of
            # tile 0)
            for i in range(4):
                nc.sync.dma_start(w2_sb[:, 0, 2 * i:2 * i + 2, :, :],
                                  w2_d[:, 0, 2 * i:2 * i + 2, :, :])

            w_late_issued = [False, False]  # w1 seg 1, w2 seg 1

            for s in range(2):
                for ti, N in enumerate(seg_tiles[s]):
                    t0 = seg_off[s] + sum(seg_tiles[s][:ti])
                    if s == 0 and ti == 0:
                        xt, cf = xt0, cf0
                    else:
                        xt = xpool.tile([P, 8, 512], dt.bfloat16, tag="x")
                        nc.sync.dma_start(xt[:, :, :N], xT_d[:, :, t0:t0 + N])
                        cf = xpool.tile([P, 512], dt.bfloat16, tag="cf")
                        nc.sync.dma_start(cf[:, :N], cf_d[:, t0:t0 + N])

                    h = hpool.tile([P, 16, 512], dt.bfloat16, tag="h")
                    for mf in range(16):
                        p1 = ps1.tile([P, 512], dt.float32, tag="p1")
                        for ko in range(8):
                            nc.tensor.matmul(
                                p1[:, :N],
                                w1_sb[:, s, mf, ko, :],
                                xt[:, ko, :N],
                                start=(ko == 0), stop=(ko == 7),
                            )
                        nc.scalar.activation(h[:, mf, :N], p1[:, :N],
                                             AF.Gelu_apprx_tanh)

                    # stage seg-1 weights behind the early seg-0 compute,
                    # off tile 0's critical DMA path
                    if s == 0 and not w_late_issued[0] and (
                            ti >= 1 or ti == len(seg_tiles[0]) - 1):
                        for i in range(2):
                            nc.sync.dma_start(
                                w1_sb[:, 1, 8 * i:8 * i + 8, :, :],
                                w1_d[:, 1, 8 * i:8 * i + 8, :, :])
                        w_late_issued[0] = True
                    elif s == 0 and not w_late_issued[1] and (
                            ti >= 2 or ti == len(seg_tiles[0]) - 1):
                        for i in range(2):
                            nc.sync.dma_start(
                                w2_sb[:, 1, 4 * i:4 * i + 4, :, :],
                                w2_d[:, 1, 4 * i:4 * i + 4, :, :])
                        w_late_issued[1] = True

                    last_tile = (s == 1 and ti == len(seg_tiles[1]) - 1)
                    for mh in range(8):
                        if last_tile and mh == 7:
                            # split the final group in two so the last
                            # multiply+DMA overlaps the second half's matmuls
                            nh = ((N // 2) + 31) // 32 * 32
                            for (c0, cn) in ((0, nh), (nh, N - nh)):
                                p2 = ps2.tile([P, 512], dt.float32, tag="p2")
                                for kf in range(16):
                                    nc.tensor.matmul(
                                        p2[:, :cn],
                                        w2_sb[:, s, mh, kf, :],
                                        h[:, kf, c0:c0 + cn],
                                        start=(kf == 0), stop=(kf == 15),
                                    )
                                ot = opool.tile([P, 512], dt.bfloat16, tag="o")
                                nc.vector.tensor_mul(ot[:, :cn], p2[:, :cn],
                                                     cf[:, c0:c0 + cn])
                                nc.sync.dma_start(
                                    y_d[:, mh, t0 + c0:t0 + c0 + cn],
                                    ot[:, :cn])
                            continue
                        p2 = ps2.tile([P, 512], dt.float32, tag="p2")
                        for kf in range(16):
                            nc.tensor.matmul(
                                p2[:, :N],
                                w2_sb[:, s, mh, kf, :],
                                h[:, kf, :N],
                                start=(kf == 0), stop=(kf == 15),
                            )
                        ot = opool.tile([P, 512], dt.bfloat16, tag="o")
                        nc.vector.tensor_mul(ot[:, :N], p2[:, :N], cf[:, :N])
                        nc.sync.dma_start(y_d[:, mh, t0:t0 + N], ot[:, :N])

                # safety for degenerate shapes: make sure seg-1 weights are
                # staged before seg 1 runs
                if s == 0:
                    if not w_late_issued[0]:
                        nc.sync.dma_start(w1_sb[:, 1], w1_d[:, 1])
                        w_late_issued[0] = True
                    if not w_late_issued[1]:
                        nc.sync.dma_start(w2_sb[:, 1], w2_d[:, 1])
                        w_late_issued[1] = True

    nc.compile()
    return nc


def kernel(hidden_states, gate_weight, choosed_experts, w1, w2):
    global LAST_RESULTS
    _ensure_axon_hooks()
    from concourse import bass_utils

    x = np.asarray(hidden_states, dtype=np.float32).reshape(T, H)
    gw = np.asarray(gate_weight, dtype=np.float32)
    ce = np.asarray(choosed_experts).astype(np.int64)
    w1 = np.asarray(w1, dtype=np.float32)
    w2 = np.asarray(w2, dtype=np.float32)

    # routing: stable sort of (token, k) pairs by expert
    flat = ce.reshape(-1)
    order = np.argsort(flat, kind="stable")
    counts = np.bincount(flat, minlength=E).astype(np.int64)
    starts = np.zeros(E + 1, dtype=np.int64)
    starts[1:] = np.cumsum(counts)

    # pair largest with smallest so each pair's total is near T*K/4
    desc = np.argsort(-counts, kind="stable")
    pairs = [(int(desc[p]), int(desc[7 - p])) for p in range(4)]
    pad32 = lambda n: max(32, -(-int(n) // 32) * 32)
    CA = pad32(counts[desc[0]])
    CB = pad32(counts[desc[4]])

    nc = _CACHE.get((CB, CA))
    if nc is None:
        nc = _build(CB, CA)
        _CACHE[(CB, CA)] = nc

    bf16 = ml_dtypes.bfloat16

    # per-expert gathered activations / coefs (shared by both cores of a pair)
    def gather(e, C):
        p = order[starts[e]:starts[e + 1]]
        t_idx = p // K
        k_idx = p % K
        n_e = len(p)
        xT = np.zeros((H, C), dtype=bf16)
        xT[:, :n_e] = x[t_idx].T
        xT = xT.reshape(8, P, C).transpose(1, 0, 2)
        coef = np.zeros((C,), dtype=np.float32)
        coef[:n_e] = gw[t_idx, k_idx]
        return xT, coef

    in_maps = [None] * NCORES
    for pi, (ea, eb) in enumerate(pairs):
        xa, ca_ = gather(ea, CA)
        xb, cb_ = gather(eb, CB)
        # seg 0 = small expert (b), seg 1 = big expert (a)
        xT = np.ascontiguousarray(np.concatenate([xb, xa], axis=2))
        coef = np.concatenate([cb_, ca_]).astype(bf16)
        coef = np.ascontiguousarray(
            np.broadcast_to(coef[None, :], (P, CB + CA)))
        for half in range(2):
            sl = slice(half * F2, (half + 1) * F2)
            # w1 blocked [p, seg, fb, ko, j]; w2 blocked [p, seg, mh, kf, j]
            w1h = np.stack([
                w1[e][:, sl].astype(bf16).reshape(8, P, 16, 128)
                .transpose(1, 2, 0, 3)
                for e in (eb, ea)], axis=1)
            w2h = np.stack([
                w2[e][sl, :].astype(bf16).reshape(16, P, 8, 128)
                .transpose(1, 2, 0, 3)
                for e in (eb, ea)], axis=1)
            in_maps[2 * pi + half] = {
                "xT": xT, "w1": np.ascontiguousarray(w1h),
                "w2": np.ascontiguousarray(w2h), "coef": coef,
            }

    res = bass_utils.run_bass_kernel_spmd(nc, in_maps, list(range(NCORES)),
                                          trace=TRACE)
    LAST_RESULTS = res

    # combine: sum the two F-half partials per pair, split back per expert,
    # then sum the K contributions per token
    ys = [None] * E
    for pi, (ea, eb) in enumerate(pairs):
        ysum = (res.results[2 * pi]["y"].astype(np.float32)
                + res.results[2 * pi + 1]["y"].astype(np.float32))
        yT = ysum.transpose(1, 0, 2).reshape(H, CB + CA)
        ys[eb] = yT[:, :counts[eb]].T
        ys[ea] = yT[:, CB:CB + counts[ea]].T
    all_pairs = np.concatenate([ys[e] for e in range(E)], axis=0)
    out_pairs = np.empty((T * K, H), dtype=np.float32)
    out_pairs[order] = all_pairs
    return out_pairs.reshape(T, K, H).sum(axis=1)
